# revision 60
# baseline (speedup 1.0000x reference)
"""Local (sliding-window causal) attention kernel for Trainium2, 8 NeuronCores.

Reference computation (per batch b, head h):
  q = x @ Wq + bq ; k = x @ Wk + bk ; v = x @ Wv + bv   (16 heads of 64)
  S = q k^T / 8, masked to the causal band  i-255 <= j <= i
  out = softmax(S) @ v

Sharding: B=2, H=16 -> each of 8 cores owns a 128-wide column slice of the
QKV projections (2 heads) for both batches. Inputs are replicated; weights
column-sliced per core; no collectives.

Scheme (fp8 DoubleRow projections, bf16 attention):
  - x ships as an fp8 pair (xh = fp8(x^T), xl = fp8(x^T - xh)) in per-chunk
    tensors (contiguous rows -> 1 DMA descriptor per partition); weights as
    fp8 pairs of 64*W (64x scaling keeps W ~N(0,0.02) in e4m3 normal range).
    Projections accumulate correction terms in PSUM via DoubleRow
    (2 k-subtiles per pass):
       64*q = xh@wq8 + xh@wql [+ xl@wq8]     (same for k; v always 3 terms)
    then a tensor_scalar copy rescales by 1/64 (+bias) into bf16 SBUF.
  - Attention per (b, key-block kb of 128): S^T for both heads lands in one
    2-bank PSUM tile; one ACT exp (scale=1/8) -> P~^T bf16; the two
    triangular 0/1 masks multiply in (diag cols 0:128, tail cols 256:384;
    the middle 128 are always in-band) on DVE or Pool. PV matmuls
    accumulate [128q, 65] per (qb, h) into per-3-qb PSUM "super" tiles
    (col 64 = row sums via the ones-column of V'); a DVE copy stages
    [128, 3*130] bf16 to SBUF, shipped unnormalized; the host divides by
    the row sums and adds bv.
"""

import sys

import numpy as np

try:
    import concourse.bass as bass  # noqa: F401
except ImportError:
    sys.path.insert(0, "/opt/trn_rl_repo")

import concourse.bass as bass  # noqa: F401
import concourse.tile as tile
from concourse import bacc, mybir
from concourse.bass_utils import run_bass_kernel_spmd

import ml_dtypes

P = 128
B, L, D = 2, 2048, 1024
NT = B * L            # 4096 tokens
KSUB = D // P         # 8 contraction subtiles (4 DoubleRow pairs)
G = 256               # DoubleRow token group (rhs free = 2*G = 512)
NLB = NT // P         # 32 token blocks
NKB = L // P          # 16 key blocks per batch
QW = 384              # query window per key block
DH = 64               # head dim
OC = 2 * (DH + 1)     # output cols per token (2 heads x (o, rowsum))
NSUP = 6              # supers per batch (3 query blocks each)
NCORES = 8
WS = 64.0             # weight pre-scale for fp8
QK_TERMS = 2          # 3 = full correction, 2 = drop xl@w8 (faster, riskier)

# (start, size) of the x chunks; first two are small to cut startup latency
CHUNKS = [(0, 256), (256, 256), (512, 512), (1024, 512), (1536, 512),
          (2048, 512), (2560, 512), (3072, 512), (3584, 512)]

F32 = mybir.dt.float32
BF16 = mybir.dt.bfloat16
FP8 = mybir.dt.float8e4

DR = mybir.MatmulPerfMode.DoubleRow


def build_program():
    nc = bacc.Bacc("TRN2", target_bir_lowering=False, debug=False,
                   num_devices=NCORES)

    xh_ds, xl_ds = [], []
    for i, (t0, sz) in enumerate(CHUNKS):
        xh_ds.append(nc.dram_tensor(f"xh{i}", [P, KSUB, sz], FP8,
                                    kind="ExternalInput").ap())
        xl_ds.append(nc.dram_tensor(f"xl{i}", [P, KSUB, sz], FP8,
                                    kind="ExternalInput").ap())
    # constants ride in two DMAs: the QK blob (4 weight tensors + biases)
    # gates the first projection; the V blob (V weights + masks) only the
    # first attend.
    QBLOB = 4 * 1024 + 2 * 4
    VBLOB = 2 * 1024 + 2 * 512
    qblob_d = nc.dram_tensor("qblob", [P, QBLOB], mybir.dt.uint8,
                             kind="ExternalInput").ap()
    vblob_d = nc.dram_tensor("vblob", [P, VBLOB], mybir.dt.uint8,
                             kind="ExternalInput").ap()
    out_d = nc.dram_tensor("out", [B, NSUP, P, 3 * OC], BF16,
                           kind="ExternalOutput").ap()

    with tile.TileContext(nc) as tc:
        with (
            tc.tile_pool(name="const", bufs=1) as const,
            tc.tile_pool(name="qkv", bufs=1) as qkv,
            tc.tile_pool(name="xhp", bufs=4) as xhp,
            tc.tile_pool(name="xlp", bufs=4) as xlp,
            tc.tile_pool(name="ptp", bufs=12) as ptp,
            tc.tile_pool(name="ostp", bufs=3) as ostp,
            tc.tile_pool(name="pjps", bufs=2, space="PSUM") as pj_ps,
            tc.tile_pool(name="pvps", bufs=1, space="PSUM") as pv_ps,
            tc.tile_pool(name="stps", bufs=3, space="PSUM") as st_ps,
            tc.tile_pool(name="ops", bufs=2, space="PSUM") as o_ps,
        ):
            qblob = const.tile([P, QBLOB], mybir.dt.uint8, tag="qblob")
            vblob = const.tile([P, VBLOB], mybir.dt.uint8, tag="vblob")
            w_sb = {}
            for wi, wn in enumerate(("wq8", "wql", "wk8", "wkl")):
                w_sb[wn] = (qblob[:, wi * 1024:(wi + 1) * 1024]
                            .bitcast(FP8)
                            .rearrange("p (k m) -> p k m", k=KSUB))
            bq_sb = qblob[:, 4096:4100].bitcast(F32)
            bk_sb = qblob[:, 4100:4104].bitcast(F32)
            for wi, wn in enumerate(("wvh", "wvl")):
                w_sb[wn] = (vblob[:, wi * 1024:(wi + 1) * 1024]
                            .bitcast(FP8)
                            .rearrange("p (k m) -> p k m", k=KSUB))
            mk2_sb = (vblob[:, 2048:3072].bitcast(BF16)
                      .rearrange("p (h r m) -> p h r m", h=2, r=2))
            mkd_sb = mk2_sb[:, :, 0, :]

            qt_sb = qkv.tile([P, NT], BF16, tag="qt")   # 2 heads' dh on parts
            kt_sb = qkv.tile([P, NT], BF16, tag="kt")
            v_sb = qkv.tile([P, 2, NLB, DH + 1], BF16, tag="v")
            nc.vector.memset(v_sb[:, :, :, DH:DH + 1], 1.0)

            EMIT = list(range(len(CHUNKS)))

            xhs, xls = {}, {}
            for j, i in enumerate(EMIT):
                sz = CHUNKS[i][1]
                xhs[i] = xhp.tile([P, KSUB, sz], FP8, tag=f"xh{j % 4}",
                                  name=f"xh{i}")
                xls[i] = xlp.tile([P, KSUB, sz], FP8, tag=f"xl{j % 4}",
                                  name=f"xl{i}")

            # xh leads xl by one chunk: Q/K only consume xh, and V (the only
            # xl consumer) is emitted one chunk behind.
            nc.sync.dma_start(qblob[:], qblob_d)
            nc.sync.dma_start(xhs[0][:], xh_ds[0])
            nc.sync.dma_start(xhs[1][:], xh_ds[1])
            nc.sync.dma_start(vblob[:], vblob_d)
            nc.sync.dma_start(xls[0][:], xl_ds[0])
            nc.sync.dma_start(xhs[2][:], xh_ds[2])
            for i in EMIT[3:]:
                nc.sync.dma_start(xhs[i][:], xh_ds[i])
                nc.sync.dma_start(xls[i - 2][:], xl_ds[i - 2])
            for i in EMIT[-2:]:
                nc.sync.dma_start(xls[i][:], xl_ds[i])

            # Two 256-col projection groups share each PSUM bank (the tile
            # tracker is region-level, and a start=True bank clear only
            # resets has_written -- finished data in the other half is
            # unaffected), giving 4 slots in 2 banks.
            pj_rot = {"tile": None, "half": 1}

            def pj_slot():
                if pj_rot["half"] == 1:
                    pj_rot["tile"] = pj_ps.tile([P, 2, G], F32, tag="pj",
                                                name="pj")
                    pj_rot["half"] = 0
                else:
                    pj_rot["half"] = 1
                return pj_rot["tile"][:, pj_rot["half"], :]

            def proj_qk(ci, lg, w8, wl, bias, dst, ceng="dve"):
                """One 256-token DoubleRow group for Q^T or K^T."""
                t0, sz = CHUNKS[ci]
                g0 = lg * G
                sl = pj_slot()
                terms = ((w8, xhs[ci]), (wl, xhs[ci]))
                if QK_TERMS == 3:
                    terms += ((w8, xls[ci]),)
                nmm = 4 * len(terms)
                n = 0
                for wt, xt in terms:
                    for kp in range(KSUB // 2):
                        nc.tensor.matmul(
                            sl, lhsT=wt[:, 2 * kp:2 * kp + 2, :],
                            rhs=xt[:, 2 * kp:2 * kp + 2, g0:g0 + G],
                            start=(n == 0), stop=(n == nmm - 1),
                            perf_mode=DR, skip_group_check=True)
                        n += 1
                if ceng == "act":
                    nc.scalar.activation(
                        dst[:, t0 + g0:t0 + g0 + G], sl,
                        mybir.ActivationFunctionType.Identity,
                        bias=bias[:, 0:1], scale=1.0 / WS)
                else:
                    nc.vector.tensor_scalar(
                        dst[:, t0 + g0:t0 + g0 + G], sl,
                        1.0 / WS, bias[:, 0:1],
                        op0=mybir.AluOpType.mult, op1=mybir.AluOpType.add)

            def proj_v(ci):
                """V for one chunk: one PSUM bank, 128-token lb blocks."""
                t0, sz = CHUNKS[ci]
                nlb = sz // P
                pv = pv_ps.tile([P, 4, P], F32, tag="pv", name="pv")
                terms = ((w_sb["wvh"], xhs[ci]), (w_sb["wvl"], xhs[ci]),
                         (w_sb["wvh"], xls[ci]))
                for lo in range(nlb):
                    n = 0
                    for wt, xt in terms:
                        for kp in range(KSUB // 2):
                            nc.tensor.matmul(
                                pv[:, lo, :],
                                lhsT=xt[:, 2 * kp:2 * kp + 2,
                                        lo * P:(lo + 1) * P],
                                rhs=wt[:, 2 * kp:2 * kp + 2, :],
                                start=(n == 0), stop=(n == 11),
                                perf_mode=DR, skip_group_check=True)
                            n += 1
                lb0 = t0 // P
                for h in range(2):
                    nc.vector.tensor_scalar_mul(
                        v_sb[:, h, lb0:lb0 + nlb, 0:DH],
                        pv[:, 0:nlb, h * DH:(h + 1) * DH], 1.0 / WS)

            o_tiles = {}
            o_done = {}

            def flush_super(b, s):
                """Copy a finished 3-qb PSUM super tile to SBUF + DMA out."""
                nslots = min(3, NKB - 3 * s)
                ot = o_tiles.pop((b, s))
                st = ostp.tile([P, 3 * OC], BF16, tag="ost",
                               name=f"ost_{b}_{s}")
                w = nslots * OC
                src_ap = ot[:, 0:nslots, :].rearrange("p s c -> p (s c)")
                if b == 1 and s >= 4:
                    nc.scalar.copy(st[:, 0:w], src_ap)
                else:
                    nc.vector.tensor_copy(st[:, 0:w], src_ap)
                nc.sync.dma_start(out_d[b, s, :, 0:w], st[:, 0:w])

            def pv_block(b, kb, h, pt_ap):
                for qb in range(kb, min(kb + 3, NKB)):
                    s, slot = divmod(qb, 3)
                    qoff = (qb - kb) * P
                    first = (kb == max(qb - 2, 0))
                    if first and slot == 0 and h == 0:
                        o_tiles[(b, s)] = o_ps.tile(
                            [P, 3, OC], F32, tag="o", name=f"o_{b}_{s}")
                    ot = o_tiles[(b, s)]
                    nc.tensor.matmul(
                        ot[:, slot, h * (DH + 1):(h + 1) * (DH + 1)],
                        lhsT=pt_ap[:, qoff:qoff + P],
                        rhs=v_sb[:, h, b * NKB + kb, :],
                        start=(first and slot == 0 and h == 0),
                        stop=(qb == kb), skip_group_check=True)
                    if qb == kb and h == 1:
                        done = o_done.get((b, s), 0) + 1
                        o_done[(b, s)] = done
                        if done == min(3, NKB - 3 * s):
                            flush_super(b, s)

            def attend_scores(b, kb, mask_engine):
                """S^T + exp + masks for one key block; PV comes later."""
                t0 = b * L
                k0 = t0 + kb * P
                qw = min(QW, L - kb * P)
                eng = nc.gpsimd if mask_engine == "pool" else nc.vector
                pt = ptp.tile([P, 2, QW], BF16, tag="pt", name="pt")
                for h in range(2):
                    hs = h * DH
                    sth = st_ps.tile([P, 512], F32, tag="st", name=f"st{h}")
                    nc.tensor.matmul(sth[:, 0:qw],
                                     lhsT=kt_sb[hs:hs + DH, k0:k0 + P],
                                     rhs=qt_sb[hs:hs + DH, k0:k0 + qw],
                                     start=True, stop=True)
                    nc.scalar.activation(
                        pt[:, h, 0:qw], sth[:, 0:qw],
                        mybir.ActivationFunctionType.Exp, scale=0.125)
                if qw == QW:
                    ptv = (pt[:, :, :].rearrange("p h (r m) -> p h r m", m=P)
                           [:, :, ::2, :])
                    eng.tensor_mul(ptv, ptv, mk2_sb[:])
                else:
                    eng.tensor_mul(pt[:, :, 0:P], pt[:, :, 0:P], mkd_sb[:])
                return pt

            def attend_pv(b, kb, pt):
                for h in range(2):
                    pv_block(b, kb, h, pt[:, h, :])

            # Attend(b, kb) is ready once Q^T/K^T cover batch-local token
            # (kb+3)*128 and V covers key block kb.  Scores run one block
            # ahead of PV so the PE never waits on the exp+mask chain of
            # the block it just scored.
            att_i = [0]
            scored = []

            def pop_ready(b, pend, q_cover, v_cover, tail=False):
                while pend:
                    kb = pend[0]
                    if (min(kb + 3, NKB) * P > q_cover
                            or (kb + 1) * P > v_cover):
                        break
                    pend.pop(0)
                    i = att_i[0]
                    att_i[0] += 1
                    eng = "pool" if 6 <= i < 16 else "dve"
                    pt = attend_scores(b, kb, eng)
                    scored.append((b, kb, pt))
                    depth = 8 if att_i[0] < 27 else 2
                    while len(scored) > depth:
                        attend_pv(*scored.pop(0))
                if tail and not pend:
                    while scored:
                        attend_pv(*scored.pop(0))

            stt = {b: {"pend": list(range(NKB)), "qc": 0, "vc": 0}
                   for b in range(B)}

            def emit_v(ci, tail=False):
                t0, sz = CHUNKS[ci]
                b = t0 // L
                s = stt[b]
                proj_v(ci)
                s["vc"] = t0 - b * L + sz
                pop_ready(b, s["pend"], s["qc"], s["vc"], tail=tail)

            prev = []
            for ci in EMIT:
                t0, sz = CHUNKS[ci]
                b = t0 // L
                s = stt[b]
                for lg in range(sz // G):
                    proj_qk(ci, lg, w_sb["wq8"], w_sb["wql"], bq_sb, qt_sb)
                    proj_qk(ci, lg, w_sb["wk8"], w_sb["wkl"], bk_sb, kt_sb)
                    s["qc"] = t0 - b * L + (lg + 1) * G
                    pop_ready(b, s["pend"], s["qc"], s["vc"])
                    if lg == 0 and len(prev) > 1:
                        emit_v(prev.pop(0))  # V lags two chunks
                prev.append(ci)
            emit_v(prev.pop(0))
            emit_v(prev.pop(0), tail=True)
            for b in range(B):
                assert not stt[b]["pend"], (b, stt[b])
    nc.finalize()
    return nc


_NC = None


def _get_nc():
    global _NC
    if _NC is None:
        _NC = build_program()
    return _NC


def _masks():
    pk = np.arange(P)[:, None]
    f = np.arange(P)[None, :]
    mkd = (f >= pk).astype(np.float32)       # diag block: query >= key
    mkt = (f < pk).astype(np.float32)        # tail block: dist <= 255
    mkd2 = np.repeat(mkd[:, None, :], 2, axis=1).astype(ml_dtypes.bfloat16)
    mkt2 = np.repeat(mkt[:, None, :], 2, axis=1).astype(ml_dtypes.bfloat16)
    return np.ascontiguousarray(mkd2), np.ascontiguousarray(mkt2)


def _prepare_in_maps(inputs):
    hs = np.asarray(inputs["hidden_states"], np.float32)
    Wq = np.asarray(inputs["Wq"], np.float32)
    Wk = np.asarray(inputs["Wk"], np.float32)
    Wv = np.asarray(inputs["Wv"], np.float32)
    bq = np.asarray(inputs["bq"], np.float32)
    bk = np.asarray(inputs["bk"], np.float32)

    x_flat = hs.reshape(NT, D)
    # xt[p, k, t] = x_flat[t, k*128+p]
    xt = np.ascontiguousarray(
        x_flat.T.reshape(KSUB, P, NT).transpose(1, 0, 2))
    xh = xt.astype(ml_dtypes.float8_e4m3)
    xl = (xt - xh.astype(np.float32)).astype(ml_dtypes.float8_e4m3)
    chunks = {}
    for i, (t0, sz) in enumerate(CHUNKS):
        chunks[f"xh{i}"] = np.ascontiguousarray(xh[:, :, t0:t0 + sz])
        chunks[f"xl{i}"] = np.ascontiguousarray(xl[:, :, t0:t0 + sz])
    mkd, mkt = _masks()

    def wsplit(W, c):
        # [P, KSUB, 128]: w[p, k, m] = WS * W[k*128+p, c*128+m]
        ws = np.ascontiguousarray(
            (WS * W[:, c * P:(c + 1) * P]).reshape(KSUB, P, P)
            .transpose(1, 0, 2))
        w8 = ws.astype(ml_dtypes.float8_e4m3)
        wl = (ws - w8.astype(np.float32)).astype(ml_dtypes.float8_e4m3)
        return w8, wl

    in_maps = []
    for c in range(NCORES):
        wq8, wql = wsplit(Wq, c)
        wk8, wkl = wsplit(Wk, c)
        wvh, wvl = wsplit(Wv, c)
        bqc = np.ascontiguousarray(bq[c * P:(c + 1) * P].reshape(P, 1))
        bkc = np.ascontiguousarray(bk[c * P:(c + 1) * P].reshape(P, 1))
        qblob = np.concatenate(
            [w.reshape(P, KSUB * P).view(np.uint8)
             for w in (wq8, wql, wk8, wkl)]
            + [bqc.view(np.uint8), bkc.view(np.uint8)], axis=1)
        mk2 = np.stack([mkd, mkt], axis=2)  # [P, 2(h), 2(r), 128]
        vblob = np.concatenate(
            [w.reshape(P, KSUB * P).view(np.uint8) for w in (wvh, wvl)]
            + [mk2.reshape(P, 4 * P).view(np.uint8)], axis=1)
        m = dict(chunks)
        m["qblob"] = np.ascontiguousarray(qblob)
        m["vblob"] = np.ascontiguousarray(vblob)
        in_maps.append(m)
    return in_maps


def run(inputs, trace=False, **kwargs):
    nc = _get_nc()
    in_maps = _prepare_in_maps(inputs)
    res = run_bass_kernel_spmd(nc, in_maps, core_ids=list(range(NCORES)),
                               trace=trace, **kwargs)
    bv = np.asarray(inputs["bv"], np.float32)
    outs = []
    for c in range(NCORES):
        o = np.asarray(res.results[c]["out"]).astype(np.float32)
        # [B, NSUP, P, 3, OC]; (s, slot) -> query block 3s+slot, row p
        o = o.reshape(B, NSUP, P, 3, OC).transpose(0, 1, 3, 2, 4)
        o = o.reshape(B, NSUP * 3 * P, OC)[:, :L]      # [B, L, OC]
        for h in range(2):
            c0 = h * (DH + 1)
            outs.append(o[:, :, c0:c0 + DH] / o[:, :, c0 + DH:c0 + DH + 1])
    full = np.concatenate(outs, axis=2)
    full = full + bv[None, None, :]
    return full.astype(np.float32), res


def kernel(**inputs):
    out, _ = run(inputs, trace=False)
    return out


# revision 61
# speedup vs baseline: 1.0023x; 1.0023x over previous
"""Local (sliding-window causal) attention kernel for Trainium2, 8 NeuronCores.

Reference computation (per batch b, head h):
  q = x @ Wq + bq ; k = x @ Wk + bk ; v = x @ Wv + bv   (16 heads of 64)
  S = q k^T / 8, masked to the causal band  i-255 <= j <= i
  out = softmax(S) @ v

Sharding: B=2, H=16 -> each of 8 cores owns a 128-wide column slice of the
QKV projections (2 heads) for both batches. Inputs are replicated; weights
column-sliced per core; no collectives.

Scheme (fp8 DoubleRow projections, bf16 attention):
  - x ships as an fp8 pair (xh = fp8(x^T), xl = fp8(x^T - xh)) in per-chunk
    tensors (contiguous rows -> 1 DMA descriptor per partition); weights as
    fp8 pairs of 64*W (64x scaling keeps W ~N(0,0.02) in e4m3 normal range).
    Projections accumulate correction terms in PSUM via DoubleRow
    (2 k-subtiles per pass):
       64*q = xh@wq8 + xh@wql [+ xl@wq8]     (same for k; v always 3 terms)
    then a tensor_scalar copy rescales by 1/64 (+bias) into bf16 SBUF.
  - Attention per (b, key-block kb of 128): S^T for both heads lands in one
    2-bank PSUM tile; one ACT exp (scale=1/8) -> P~^T bf16; the two
    triangular 0/1 masks multiply in (diag cols 0:128, tail cols 256:384;
    the middle 128 are always in-band) on DVE or Pool. PV matmuls
    accumulate [128q, 65] per (qb, h) into per-3-qb PSUM "super" tiles
    (col 64 = row sums via the ones-column of V'); a DVE copy stages
    [128, 3*130] bf16 to SBUF, shipped unnormalized; the host divides by
    the row sums and adds bv.
"""

import sys

import numpy as np

try:
    import concourse.bass as bass  # noqa: F401
except ImportError:
    sys.path.insert(0, "/opt/trn_rl_repo")

import concourse.bass as bass  # noqa: F401
import concourse.tile as tile
from concourse import bacc, mybir
from concourse.bass_utils import run_bass_kernel_spmd

import ml_dtypes

P = 128
B, L, D = 2, 2048, 1024
NT = B * L            # 4096 tokens
KSUB = D // P         # 8 contraction subtiles (4 DoubleRow pairs)
G = 256               # DoubleRow token group (rhs free = 2*G = 512)
NLB = NT // P         # 32 token blocks
NKB = L // P          # 16 key blocks per batch
QW = 384              # query window per key block
DH = 64               # head dim
OC = 2 * (DH + 1)     # output cols per token (2 heads x (o, rowsum))
NSUP = 6              # supers per batch (3 query blocks each)
NCORES = 8
WS = 64.0             # weight pre-scale for fp8
QK_TERMS = 2          # 3 = full correction, 2 = drop xl@w8 (faster, riskier)

# (start, size) of the x chunks; first two are small to cut startup latency
CHUNKS = [(0, 256), (256, 256), (512, 512), (1024, 512), (1536, 512),
          (2048, 512), (2560, 512), (3072, 512), (3584, 512)]

F32 = mybir.dt.float32
BF16 = mybir.dt.bfloat16
FP8 = mybir.dt.float8e4

DR = mybir.MatmulPerfMode.DoubleRow


def build_program():
    nc = bacc.Bacc("TRN2", target_bir_lowering=False, debug=False,
                   num_devices=NCORES)

    xh_ds, xl_ds = [], []
    for i, (t0, sz) in enumerate(CHUNKS):
        xh_ds.append(nc.dram_tensor(f"xh{i}", [P, KSUB, sz], FP8,
                                    kind="ExternalInput").ap())
        xl_ds.append(nc.dram_tensor(f"xl{i}", [P, KSUB, sz], FP8,
                                    kind="ExternalInput").ap())
    # constants ride in two DMAs: the QK blob (4 weight tensors + biases)
    # gates the first projection; the V blob (V weights + masks) only the
    # first attend.
    QBLOB = 4 * 1024 + 2 * 4
    VBLOB = 2 * 1024 + 2 * 512
    qblob_d = nc.dram_tensor("qblob", [P, QBLOB], mybir.dt.uint8,
                             kind="ExternalInput").ap()
    vblob_d = nc.dram_tensor("vblob", [P, VBLOB], mybir.dt.uint8,
                             kind="ExternalInput").ap()
    out_d = nc.dram_tensor("out", [B, NSUP, P, 3 * OC], BF16,
                           kind="ExternalOutput").ap()

    with tile.TileContext(nc) as tc:
        with (
            tc.tile_pool(name="const", bufs=1) as const,
            tc.tile_pool(name="qkv", bufs=1) as qkv,
            tc.tile_pool(name="xhp", bufs=4) as xhp,
            tc.tile_pool(name="xlp", bufs=4) as xlp,
            tc.tile_pool(name="ptp", bufs=12) as ptp,
            tc.tile_pool(name="ostp", bufs=3) as ostp,
            tc.tile_pool(name="pjps", bufs=2, space="PSUM") as pj_ps,
            tc.tile_pool(name="pvps", bufs=1, space="PSUM") as pv_ps,
            tc.tile_pool(name="stps", bufs=3, space="PSUM") as st_ps,
            tc.tile_pool(name="ops", bufs=2, space="PSUM") as o_ps,
        ):
            qblob = const.tile([P, QBLOB], mybir.dt.uint8, tag="qblob")
            vblob = const.tile([P, VBLOB], mybir.dt.uint8, tag="vblob")
            w_sb = {}
            for wi, wn in enumerate(("wq8", "wql", "wk8", "wkl")):
                w_sb[wn] = (qblob[:, wi * 1024:(wi + 1) * 1024]
                            .bitcast(FP8)
                            .rearrange("p (k m) -> p k m", k=KSUB))
            bq_sb = qblob[:, 4096:4100].bitcast(F32)
            bk_sb = qblob[:, 4100:4104].bitcast(F32)
            for wi, wn in enumerate(("wvh", "wvl")):
                w_sb[wn] = (vblob[:, wi * 1024:(wi + 1) * 1024]
                            .bitcast(FP8)
                            .rearrange("p (k m) -> p k m", k=KSUB))
            mkd_sb = (vblob[:, 2048:2560].bitcast(BF16)
                      .rearrange("p (h m) -> p h m", h=2))
            mkt_sb = (vblob[:, 2560:3072].bitcast(BF16)
                      .rearrange("p (h m) -> p h m", h=2))

            qt_sb = qkv.tile([P, NT], BF16, tag="qt")   # 2 heads' dh on parts
            kt_sb = qkv.tile([P, NT], BF16, tag="kt")
            v_sb = qkv.tile([P, 2, NLB, DH + 1], BF16, tag="v")
            nc.vector.memset(v_sb[:, :, :, DH:DH + 1], 1.0)

            EMIT = list(range(len(CHUNKS)))

            xhs, xls = {}, {}
            for j, i in enumerate(EMIT):
                sz = CHUNKS[i][1]
                xhs[i] = xhp.tile([P, KSUB, sz], FP8, tag=f"xh{j % 4}",
                                  name=f"xh{i}")
                xls[i] = xlp.tile([P, KSUB, sz], FP8, tag=f"xl{j % 4}",
                                  name=f"xl{i}")

            # xh leads xl by one chunk: Q/K only consume xh, and V (the only
            # xl consumer) is emitted one chunk behind.
            nc.sync.dma_start(qblob[:], qblob_d)
            nc.sync.dma_start(xhs[0][:], xh_ds[0])
            nc.sync.dma_start(xhs[1][:], xh_ds[1])
            nc.sync.dma_start(vblob[:], vblob_d)
            nc.sync.dma_start(xls[0][:], xl_ds[0])
            nc.sync.dma_start(xhs[2][:], xh_ds[2])
            for i in EMIT[3:]:
                nc.sync.dma_start(xhs[i][:], xh_ds[i])
                nc.sync.dma_start(xls[i - 2][:], xl_ds[i - 2])
            for i in EMIT[-2:]:
                nc.sync.dma_start(xls[i][:], xl_ds[i])

            # Two 256-col projection groups share each PSUM bank (the tile
            # tracker is region-level, and a start=True bank clear only
            # resets has_written -- finished data in the other half is
            # unaffected), giving 4 slots in 2 banks.
            pj_rot = {"tile": None, "half": 1}

            def pj_slot():
                if pj_rot["half"] == 1:
                    pj_rot["tile"] = pj_ps.tile([P, 2, G], F32, tag="pj",
                                                name="pj")
                    pj_rot["half"] = 0
                else:
                    pj_rot["half"] = 1
                return pj_rot["tile"][:, pj_rot["half"], :]

            def proj_qk(ci, lg, w8, wl, bias, dst, ceng="dve"):
                """One 256-token DoubleRow group for Q^T or K^T."""
                t0, sz = CHUNKS[ci]
                g0 = lg * G
                sl = pj_slot()
                terms = ((w8, xhs[ci]), (wl, xhs[ci]))
                if QK_TERMS == 3:
                    terms += ((w8, xls[ci]),)
                nmm = 4 * len(terms)
                n = 0
                for wt, xt in terms:
                    for kp in range(KSUB // 2):
                        nc.tensor.matmul(
                            sl, lhsT=wt[:, 2 * kp:2 * kp + 2, :],
                            rhs=xt[:, 2 * kp:2 * kp + 2, g0:g0 + G],
                            start=(n == 0), stop=(n == nmm - 1),
                            perf_mode=DR, skip_group_check=True)
                        n += 1
                if ceng == "act":
                    nc.scalar.activation(
                        dst[:, t0 + g0:t0 + g0 + G], sl,
                        mybir.ActivationFunctionType.Identity,
                        bias=bias[:, 0:1], scale=1.0 / WS)
                else:
                    nc.vector.tensor_scalar(
                        dst[:, t0 + g0:t0 + g0 + G], sl,
                        1.0 / WS, bias[:, 0:1],
                        op0=mybir.AluOpType.mult, op1=mybir.AluOpType.add)

            def proj_v(ci):
                """V for one chunk: one PSUM bank, 128-token lb blocks."""
                t0, sz = CHUNKS[ci]
                nlb = sz // P
                pv = pv_ps.tile([P, 4, P], F32, tag="pv", name="pv")
                terms = ((w_sb["wvh"], xhs[ci]), (w_sb["wvl"], xhs[ci]),
                         (w_sb["wvh"], xls[ci]))
                for lo in range(nlb):
                    n = 0
                    for wt, xt in terms:
                        for kp in range(KSUB // 2):
                            nc.tensor.matmul(
                                pv[:, lo, :],
                                lhsT=xt[:, 2 * kp:2 * kp + 2,
                                        lo * P:(lo + 1) * P],
                                rhs=wt[:, 2 * kp:2 * kp + 2, :],
                                start=(n == 0), stop=(n == 11),
                                perf_mode=DR, skip_group_check=True)
                            n += 1
                lb0 = t0 // P
                for h in range(2):
                    nc.vector.tensor_scalar_mul(
                        v_sb[:, h, lb0:lb0 + nlb, 0:DH],
                        pv[:, 0:nlb, h * DH:(h + 1) * DH], 1.0 / WS)

            o_tiles = {}
            o_done = {}

            def flush_super(b, s):
                """Copy a finished 3-qb PSUM super tile to SBUF + DMA out."""
                nslots = min(3, NKB - 3 * s)
                ot = o_tiles.pop((b, s))
                st = ostp.tile([P, 3 * OC], BF16, tag="ost",
                               name=f"ost_{b}_{s}")
                w = nslots * OC
                src_ap = ot[:, 0:nslots, :].rearrange("p s c -> p (s c)")
                if b == 1 and s >= 4:
                    nc.scalar.copy(st[:, 0:w], src_ap)
                else:
                    nc.vector.tensor_copy(st[:, 0:w], src_ap)
                nc.sync.dma_start(out_d[b, s, :, 0:w], st[:, 0:w])

            def pv_block(b, kb, h, pt_ap):
                for qb in range(kb, min(kb + 3, NKB)):
                    s, slot = divmod(qb, 3)
                    qoff = (qb - kb) * P
                    first = (kb == max(qb - 2, 0))
                    if first and slot == 0 and h == 0:
                        o_tiles[(b, s)] = o_ps.tile(
                            [P, 3, OC], F32, tag="o", name=f"o_{b}_{s}")
                    ot = o_tiles[(b, s)]
                    nc.tensor.matmul(
                        ot[:, slot, h * (DH + 1):(h + 1) * (DH + 1)],
                        lhsT=pt_ap[:, qoff:qoff + P],
                        rhs=v_sb[:, h, b * NKB + kb, :],
                        start=(first and slot == 0 and h == 0),
                        stop=(qb == kb), skip_group_check=True)
                    if qb == kb and h == 1:
                        done = o_done.get((b, s), 0) + 1
                        o_done[(b, s)] = done
                        if done == min(3, NKB - 3 * s):
                            flush_super(b, s)

            def attend_scores(b, kb, mask_engine):
                """S^T + exp + masks for one key block; PV comes later."""
                t0 = b * L
                k0 = t0 + kb * P
                qw = min(QW, L - kb * P)
                eng = nc.gpsimd if mask_engine == "pool" else nc.vector
                pt = ptp.tile([P, 2, QW], BF16, tag="pt", name="pt")
                for h in range(2):
                    hs = h * DH
                    sth = st_ps.tile([P, 512], F32, tag="st", name=f"st{h}")
                    nc.tensor.matmul(sth[:, 0:qw],
                                     lhsT=kt_sb[hs:hs + DH, k0:k0 + P],
                                     rhs=qt_sb[hs:hs + DH, k0:k0 + qw],
                                     start=True, stop=True)
                    nc.scalar.activation(
                        pt[:, h, 0:qw], sth[:, 0:qw],
                        mybir.ActivationFunctionType.Exp, scale=0.125)
                eng.tensor_mul(pt[:, :, 0:P], pt[:, :, 0:P], mkd_sb[:])
                if qw == QW:
                    eng.tensor_mul(pt[:, :, 2 * P:3 * P],
                                   pt[:, :, 2 * P:3 * P], mkt_sb[:])
                return pt

            def attend_pv(b, kb, pt):
                for h in range(2):
                    pv_block(b, kb, h, pt[:, h, :])

            # Attend(b, kb) is ready once Q^T/K^T cover batch-local token
            # (kb+3)*128 and V covers key block kb.  Scores run one block
            # ahead of PV so the PE never waits on the exp+mask chain of
            # the block it just scored.
            att_i = [0]
            scored = []

            def pop_ready(b, pend, q_cover, v_cover, tail=False):
                while pend:
                    kb = pend[0]
                    if (min(kb + 3, NKB) * P > q_cover
                            or (kb + 1) * P > v_cover):
                        break
                    pend.pop(0)
                    i = att_i[0]
                    att_i[0] += 1
                    eng = "pool" if 6 <= i < 16 else "dve"
                    pt = attend_scores(b, kb, eng)
                    scored.append((b, kb, pt))
                    depth = 8 if att_i[0] < 27 else 2
                    while len(scored) > depth:
                        attend_pv(*scored.pop(0))
                if tail and not pend:
                    while scored:
                        attend_pv(*scored.pop(0))

            stt = {b: {"pend": list(range(NKB)), "qc": 0, "vc": 0}
                   for b in range(B)}

            def emit_v(ci, tail=False):
                t0, sz = CHUNKS[ci]
                b = t0 // L
                s = stt[b]
                proj_v(ci)
                s["vc"] = t0 - b * L + sz
                pop_ready(b, s["pend"], s["qc"], s["vc"], tail=tail)

            prev = []
            for ci in EMIT:
                t0, sz = CHUNKS[ci]
                b = t0 // L
                s = stt[b]
                for lg in range(sz // G):
                    proj_qk(ci, lg, w_sb["wq8"], w_sb["wql"], bq_sb, qt_sb)
                    proj_qk(ci, lg, w_sb["wk8"], w_sb["wkl"], bk_sb, kt_sb)
                    s["qc"] = t0 - b * L + (lg + 1) * G
                    pop_ready(b, s["pend"], s["qc"], s["vc"])
                    if lg == 0 and len(prev) > 1:
                        emit_v(prev.pop(0))  # V lags two chunks
                prev.append(ci)
            emit_v(prev.pop(0))
            emit_v(prev.pop(0), tail=True)
            for b in range(B):
                assert not stt[b]["pend"], (b, stt[b])
    nc.finalize()
    return nc


_NC = None


def _get_nc():
    global _NC
    if _NC is None:
        _NC = build_program()
    return _NC


def _masks():
    pk = np.arange(P)[:, None]
    f = np.arange(P)[None, :]
    mkd = (f >= pk).astype(np.float32)       # diag block: query >= key
    mkt = (f < pk).astype(np.float32)        # tail block: dist <= 255
    mkd2 = np.repeat(mkd[:, None, :], 2, axis=1).astype(ml_dtypes.bfloat16)
    mkt2 = np.repeat(mkt[:, None, :], 2, axis=1).astype(ml_dtypes.bfloat16)
    return np.ascontiguousarray(mkd2), np.ascontiguousarray(mkt2)


def _prepare_in_maps(inputs):
    hs = np.asarray(inputs["hidden_states"], np.float32)
    Wq = np.asarray(inputs["Wq"], np.float32)
    Wk = np.asarray(inputs["Wk"], np.float32)
    Wv = np.asarray(inputs["Wv"], np.float32)
    bq = np.asarray(inputs["bq"], np.float32)
    bk = np.asarray(inputs["bk"], np.float32)

    x_flat = hs.reshape(NT, D)
    # xt[p, k, t] = x_flat[t, k*128+p]
    xt = np.ascontiguousarray(
        x_flat.T.reshape(KSUB, P, NT).transpose(1, 0, 2))
    xh = xt.astype(ml_dtypes.float8_e4m3)
    xl = (xt - xh.astype(np.float32)).astype(ml_dtypes.float8_e4m3)
    chunks = {}
    for i, (t0, sz) in enumerate(CHUNKS):
        chunks[f"xh{i}"] = np.ascontiguousarray(xh[:, :, t0:t0 + sz])
        chunks[f"xl{i}"] = np.ascontiguousarray(xl[:, :, t0:t0 + sz])
    mkd, mkt = _masks()

    def wsplit(W, c):
        # [P, KSUB, 128]: w[p, k, m] = WS * W[k*128+p, c*128+m]
        ws = np.ascontiguousarray(
            (WS * W[:, c * P:(c + 1) * P]).reshape(KSUB, P, P)
            .transpose(1, 0, 2))
        w8 = ws.astype(ml_dtypes.float8_e4m3)
        wl = (ws - w8.astype(np.float32)).astype(ml_dtypes.float8_e4m3)
        return w8, wl

    in_maps = []
    for c in range(NCORES):
        wq8, wql = wsplit(Wq, c)
        wk8, wkl = wsplit(Wk, c)
        wvh, wvl = wsplit(Wv, c)
        bqc = np.ascontiguousarray(bq[c * P:(c + 1) * P].reshape(P, 1))
        bkc = np.ascontiguousarray(bk[c * P:(c + 1) * P].reshape(P, 1))
        qblob = np.concatenate(
            [w.reshape(P, KSUB * P).view(np.uint8)
             for w in (wq8, wql, wk8, wkl)]
            + [bqc.view(np.uint8), bkc.view(np.uint8)], axis=1)
        vblob = np.concatenate(
            [w.reshape(P, KSUB * P).view(np.uint8) for w in (wvh, wvl)]
            + [mkd.reshape(P, 2 * P).view(np.uint8),
               mkt.reshape(P, 2 * P).view(np.uint8)], axis=1)
        m = dict(chunks)
        m["qblob"] = np.ascontiguousarray(qblob)
        m["vblob"] = np.ascontiguousarray(vblob)
        in_maps.append(m)
    return in_maps


def run(inputs, trace=False, **kwargs):
    nc = _get_nc()
    in_maps = _prepare_in_maps(inputs)
    res = run_bass_kernel_spmd(nc, in_maps, core_ids=list(range(NCORES)),
                               trace=trace, **kwargs)
    bv = np.asarray(inputs["bv"], np.float32)
    outs = []
    for c in range(NCORES):
        o = np.asarray(res.results[c]["out"]).astype(np.float32)
        # [B, NSUP, P, 3, OC]; (s, slot) -> query block 3s+slot, row p
        o = o.reshape(B, NSUP, P, 3, OC).transpose(0, 1, 3, 2, 4)
        o = o.reshape(B, NSUP * 3 * P, OC)[:, :L]      # [B, L, OC]
        for h in range(2):
            c0 = h * (DH + 1)
            outs.append(o[:, :, c0:c0 + DH] / o[:, :, c0 + DH:c0 + DH + 1])
    full = np.concatenate(outs, axis=2)
    full = full + bv[None, None, :]
    return full.astype(np.float32), res


def kernel(**inputs):
    out, _ = run(inputs, trace=False)
    return out


# revision 63
# speedup vs baseline: 1.0055x; 1.0032x over previous
"""Local (sliding-window causal) attention kernel for Trainium2, 8 NeuronCores.

Reference computation (per batch b, head h):
  q = x @ Wq + bq ; k = x @ Wk + bk ; v = x @ Wv + bv   (16 heads of 64)
  S = q k^T / 8, masked to the causal band  i-255 <= j <= i
  out = softmax(S) @ v

Sharding: B=2, H=16 -> each of 8 cores owns a 128-wide column slice of the
QKV projections (2 heads) for both batches. Inputs are replicated; weights
column-sliced per core; no collectives.

Scheme (fp8 DoubleRow projections, bf16 attention):
  - x ships as an fp8 pair (xh = fp8(x^T), xl = fp8(x^T - xh)) in per-chunk
    tensors (contiguous rows -> 1 DMA descriptor per partition); weights as
    fp8 pairs of 64*W (64x scaling keeps W ~N(0,0.02) in e4m3 normal range).
    Projections accumulate correction terms in PSUM via DoubleRow
    (2 k-subtiles per pass):
       64*q = xh@wq8 + xh@wql [+ xl@wq8]     (same for k; v always 3 terms)
    then a tensor_scalar copy rescales by 1/64 (+bias) into bf16 SBUF.
  - Attention per (b, key-block kb of 128): S^T for both heads lands in one
    2-bank PSUM tile; one ACT exp (scale=1/8) -> P~^T bf16; the two
    triangular 0/1 masks multiply in (diag cols 0:128, tail cols 256:384;
    the middle 128 are always in-band) on DVE or Pool. PV matmuls
    accumulate [128q, 65] per (qb, h) into per-3-qb PSUM "super" tiles
    (col 64 = row sums via the ones-column of V'); a DVE copy stages
    [128, 3*130] bf16 to SBUF, shipped unnormalized; the host divides by
    the row sums and adds bv.
"""

import sys

import numpy as np

try:
    import concourse.bass as bass  # noqa: F401
except ImportError:
    sys.path.insert(0, "/opt/trn_rl_repo")

import concourse.bass as bass  # noqa: F401
import concourse.tile as tile
from concourse import bacc, mybir
from concourse.bass_utils import run_bass_kernel_spmd

import ml_dtypes

P = 128
B, L, D = 2, 2048, 1024
NT = B * L            # 4096 tokens
KSUB = D // P         # 8 contraction subtiles (4 DoubleRow pairs)
G = 256               # DoubleRow token group (rhs free = 2*G = 512)
NLB = NT // P         # 32 token blocks
NKB = L // P          # 16 key blocks per batch
QW = 384              # query window per key block
DH = 64               # head dim
OC = 2 * (DH + 1)     # output cols per token (2 heads x (o, rowsum))
NSUP = 6              # supers per batch (3 query blocks each)
NCORES = 8
WS = 64.0             # weight pre-scale for fp8
QK_TERMS = 2          # 3 = full correction, 2 = drop xl@w8 (faster, riskier)

# (start, size) of the x chunks; first two are small to cut startup latency
CHUNKS = [(0, 256), (256, 256), (512, 512), (1024, 512), (1536, 512),
          (2048, 512), (2560, 512), (3072, 512), (3584, 512)]

F32 = mybir.dt.float32
BF16 = mybir.dt.bfloat16
FP8 = mybir.dt.float8e4

DR = mybir.MatmulPerfMode.DoubleRow


def build_program():
    nc = bacc.Bacc("TRN2", target_bir_lowering=False, debug=False,
                   num_devices=NCORES)

    xh_ds, xl_ds = [], []
    for i, (t0, sz) in enumerate(CHUNKS):
        xh_ds.append(nc.dram_tensor(f"xh{i}", [P, KSUB, sz], FP8,
                                    kind="ExternalInput").ap())
        xl_ds.append(nc.dram_tensor(f"xl{i}", [P, KSUB, sz], FP8,
                                    kind="ExternalInput").ap())
    # constants ride in two DMAs: the QK blob (4 weight tensors + biases)
    # gates the first projection; the V blob (V weights + masks) only the
    # first attend.
    QBLOB = 4 * 1024 + 2 * 4
    VBLOB = 2 * 1024 + 2 * 512
    qblob_d = nc.dram_tensor("qblob", [P, QBLOB], mybir.dt.uint8,
                             kind="ExternalInput").ap()
    vblob_d = nc.dram_tensor("vblob", [P, VBLOB], mybir.dt.uint8,
                             kind="ExternalInput").ap()
    out_d = nc.dram_tensor("out", [B, NSUP, P, 3 * OC], BF16,
                           kind="ExternalOutput").ap()

    with tile.TileContext(nc) as tc:
        with (
            tc.tile_pool(name="const", bufs=1) as const,
            tc.tile_pool(name="qkv", bufs=1) as qkv,
            tc.tile_pool(name="xhp", bufs=4) as xhp,
            tc.tile_pool(name="xlp", bufs=4) as xlp,
            tc.tile_pool(name="ptp", bufs=12) as ptp,
            tc.tile_pool(name="ostp", bufs=3) as ostp,
            tc.tile_pool(name="pjps", bufs=2, space="PSUM") as pj_ps,
            tc.tile_pool(name="pvps", bufs=1, space="PSUM") as pv_ps,
            tc.tile_pool(name="stps", bufs=3, space="PSUM") as st_ps,
            tc.tile_pool(name="ops", bufs=2, space="PSUM") as o_ps,
        ):
            qblob = const.tile([P, QBLOB], mybir.dt.uint8, tag="qblob")
            vblob = const.tile([P, VBLOB], mybir.dt.uint8, tag="vblob")
            w_sb = {}
            for wi, wn in enumerate(("wq8", "wql", "wk8", "wkl")):
                w_sb[wn] = (qblob[:, wi * 1024:(wi + 1) * 1024]
                            .bitcast(FP8)
                            .rearrange("p (k m) -> p k m", k=KSUB))
            bq_sb = qblob[:, 4096:4100].bitcast(F32)
            bk_sb = qblob[:, 4100:4104].bitcast(F32)
            for wi, wn in enumerate(("wvh", "wvl")):
                w_sb[wn] = (vblob[:, wi * 1024:(wi + 1) * 1024]
                            .bitcast(FP8)
                            .rearrange("p (k m) -> p k m", k=KSUB))
            mkd_sb = (vblob[:, 2048:2560].bitcast(BF16)
                      .rearrange("p (h m) -> p h m", h=2))
            mkt_sb = (vblob[:, 2560:3072].bitcast(BF16)
                      .rearrange("p (h m) -> p h m", h=2))

            qt_sb = qkv.tile([P, NT], BF16, tag="qt")   # 2 heads' dh on parts
            kt_sb = qkv.tile([P, NT], BF16, tag="kt")
            v_sb = qkv.tile([P, 2, NLB, DH + 1], BF16, tag="v")
            nc.vector.memset(v_sb[:, :, :, DH:DH + 1], 1.0)

            EMIT = list(range(len(CHUNKS)))

            xhs, xls = {}, {}
            for j, i in enumerate(EMIT):
                sz = CHUNKS[i][1]
                xhs[i] = xhp.tile([P, KSUB, sz], FP8, tag=f"xh{j % 4}",
                                  name=f"xh{i}")
                xls[i] = xlp.tile([P, KSUB, sz], FP8, tag=f"xl{j % 4}",
                                  name=f"xl{i}")

            # xh leads xl by one chunk: Q/K only consume xh, and V (the only
            # xl consumer) is emitted one chunk behind.
            nc.sync.dma_start(qblob[:], qblob_d)
            nc.sync.dma_start(xhs[0][:], xh_ds[0])
            nc.sync.dma_start(xhs[1][:], xh_ds[1])
            nc.sync.dma_start(vblob[:], vblob_d)
            nc.sync.dma_start(xls[0][:], xl_ds[0])
            nc.sync.dma_start(xhs[2][:], xh_ds[2])
            for i in EMIT[3:]:
                nc.sync.dma_start(xhs[i][:], xh_ds[i])
                nc.sync.dma_start(xls[i - 2][:], xl_ds[i - 2])
            for i in EMIT[-2:]:
                nc.sync.dma_start(xls[i][:], xl_ds[i])

            # Two 256-col projection groups share each PSUM bank (the tile
            # tracker is region-level, and a start=True bank clear only
            # resets has_written -- finished data in the other half is
            # unaffected), giving 4 slots in 2 banks.
            pj_rot = {"tile": None, "half": 1}

            def pj_slot():
                if pj_rot["half"] == 1:
                    pj_rot["tile"] = pj_ps.tile([P, 2, G], F32, tag="pj",
                                                name="pj")
                    pj_rot["half"] = 0
                else:
                    pj_rot["half"] = 1
                return pj_rot["tile"][:, pj_rot["half"], :]

            def qk_mms(sl, ci, lg, w8, wl):
                t0, sz = CHUNKS[ci]
                g0 = lg * G
                terms = ((w8, xhs[ci]), (wl, xhs[ci]))
                if QK_TERMS == 3:
                    terms += ((w8, xls[ci]),)
                nmm = 4 * len(terms)
                n = 0
                for wt, xt in terms:
                    for kp in range(KSUB // 2):
                        nc.tensor.matmul(
                            sl, lhsT=wt[:, 2 * kp:2 * kp + 2, :],
                            rhs=xt[:, 2 * kp:2 * kp + 2, g0:g0 + G],
                            start=(n == 0), stop=(n == nmm - 1),
                            perf_mode=DR, skip_group_check=True)
                        n += 1

            def proj_qk2(ci, w8, wl, bias, dst):
                """Both 256-token groups of a 512 chunk; single copy."""
                t0, sz = CHUNKS[ci]
                for lg in range(2):
                    qk_mms(pj_slot(), ci, lg, w8, wl)
                full = pj_rot["tile"][:, :, :].rearrange("p a b -> p (a b)")
                nc.vector.tensor_scalar(
                    dst[:, t0:t0 + 2 * G], full,
                    1.0 / WS, bias[:, 0:1],
                    op0=mybir.AluOpType.mult, op1=mybir.AluOpType.add)

            def proj_qk(ci, lg, w8, wl, bias, dst, ceng="dve"):
                """One 256-token DoubleRow group for Q^T or K^T."""
                t0, sz = CHUNKS[ci]
                g0 = lg * G
                sl = pj_slot()
                terms = ((w8, xhs[ci]), (wl, xhs[ci]))
                if QK_TERMS == 3:
                    terms += ((w8, xls[ci]),)
                nmm = 4 * len(terms)
                n = 0
                for wt, xt in terms:
                    for kp in range(KSUB // 2):
                        nc.tensor.matmul(
                            sl, lhsT=wt[:, 2 * kp:2 * kp + 2, :],
                            rhs=xt[:, 2 * kp:2 * kp + 2, g0:g0 + G],
                            start=(n == 0), stop=(n == nmm - 1),
                            perf_mode=DR, skip_group_check=True)
                        n += 1
                if ceng == "act":
                    nc.scalar.activation(
                        dst[:, t0 + g0:t0 + g0 + G], sl,
                        mybir.ActivationFunctionType.Identity,
                        bias=bias[:, 0:1], scale=1.0 / WS)
                else:
                    nc.vector.tensor_scalar(
                        dst[:, t0 + g0:t0 + g0 + G], sl,
                        1.0 / WS, bias[:, 0:1],
                        op0=mybir.AluOpType.mult, op1=mybir.AluOpType.add)

            def proj_v(ci):
                """V for one chunk: one PSUM bank, 128-token lb blocks."""
                t0, sz = CHUNKS[ci]
                nlb = sz // P
                pv = pv_ps.tile([P, 4, P], F32, tag="pv", name="pv")
                terms = ((w_sb["wvh"], xhs[ci]), (w_sb["wvl"], xhs[ci]),
                         (w_sb["wvh"], xls[ci]))
                for lo in range(nlb):
                    n = 0
                    for wt, xt in terms:
                        for kp in range(KSUB // 2):
                            nc.tensor.matmul(
                                pv[:, lo, :],
                                lhsT=xt[:, 2 * kp:2 * kp + 2,
                                        lo * P:(lo + 1) * P],
                                rhs=wt[:, 2 * kp:2 * kp + 2, :],
                                start=(n == 0), stop=(n == 11),
                                perf_mode=DR, skip_group_check=True)
                            n += 1
                lb0 = t0 // P
                for h in range(2):
                    nc.vector.tensor_scalar_mul(
                        v_sb[:, h, lb0:lb0 + nlb, 0:DH],
                        pv[:, 0:nlb, h * DH:(h + 1) * DH], 1.0 / WS)

            o_tiles = {}
            o_done = {}

            def flush_super(b, s):
                """Copy a finished 3-qb PSUM super tile to SBUF + DMA out."""
                nslots = min(3, NKB - 3 * s)
                ot = o_tiles.pop((b, s))
                st = ostp.tile([P, 3 * OC], BF16, tag="ost",
                               name=f"ost_{b}_{s}")
                w = nslots * OC
                src_ap = ot[:, 0:nslots, :].rearrange("p s c -> p (s c)")
                if b == 1 and s >= 4:
                    nc.scalar.copy(st[:, 0:w], src_ap)
                else:
                    nc.vector.tensor_copy(st[:, 0:w], src_ap)
                nc.sync.dma_start(out_d[b, s, :, 0:w], st[:, 0:w])

            def pv_block(b, kb, h, pt_ap):
                for qb in range(kb, min(kb + 3, NKB)):
                    s, slot = divmod(qb, 3)
                    qoff = (qb - kb) * P
                    first = (kb == max(qb - 2, 0))
                    if first and slot == 0 and h == 0:
                        o_tiles[(b, s)] = o_ps.tile(
                            [P, 3, OC], F32, tag="o", name=f"o_{b}_{s}")
                    ot = o_tiles[(b, s)]
                    nc.tensor.matmul(
                        ot[:, slot, h * (DH + 1):(h + 1) * (DH + 1)],
                        lhsT=pt_ap[:, qoff:qoff + P],
                        rhs=v_sb[:, h, b * NKB + kb, :],
                        start=(first and slot == 0 and h == 0),
                        stop=(qb == kb), skip_group_check=True)
                    if qb == kb and h == 1:
                        done = o_done.get((b, s), 0) + 1
                        o_done[(b, s)] = done
                        if done == min(3, NKB - 3 * s):
                            flush_super(b, s)

            def attend_scores(b, kb, mask_engine):
                """S^T + exp + masks for one key block; PV comes later."""
                t0 = b * L
                k0 = t0 + kb * P
                qw = min(QW, L - kb * P)
                eng = nc.gpsimd if mask_engine == "pool" else nc.vector
                pt = ptp.tile([P, 2, QW], BF16, tag="pt", name="pt")
                for h in range(2):
                    hs = h * DH
                    sth = st_ps.tile([P, 512], F32, tag="st", name=f"st{h}")
                    nc.tensor.matmul(sth[:, 0:qw],
                                     lhsT=kt_sb[hs:hs + DH, k0:k0 + P],
                                     rhs=qt_sb[hs:hs + DH, k0:k0 + qw],
                                     start=True, stop=True)
                    nc.scalar.activation(
                        pt[:, h, 0:qw], sth[:, 0:qw],
                        mybir.ActivationFunctionType.Exp, scale=0.125)
                eng.tensor_mul(pt[:, :, 0:P], pt[:, :, 0:P], mkd_sb[:])
                if qw == QW:
                    eng.tensor_mul(pt[:, :, 2 * P:3 * P],
                                   pt[:, :, 2 * P:3 * P], mkt_sb[:])
                return pt

            def attend_pv(b, kb, pt):
                for h in range(2):
                    pv_block(b, kb, h, pt[:, h, :])

            # Attend(b, kb) is ready once Q^T/K^T cover batch-local token
            # (kb+3)*128 and V covers key block kb.  Scores run one block
            # ahead of PV so the PE never waits on the exp+mask chain of
            # the block it just scored.
            att_i = [0]
            scored = []

            def pop_ready(b, pend, q_cover, v_cover, tail=False):
                while pend:
                    kb = pend[0]
                    if (min(kb + 3, NKB) * P > q_cover
                            or (kb + 1) * P > v_cover):
                        break
                    pend.pop(0)
                    i = att_i[0]
                    att_i[0] += 1
                    eng = "pool" if 6 <= i < 16 else "dve"
                    pt = attend_scores(b, kb, eng)
                    scored.append((b, kb, pt))
                    depth = 8 if att_i[0] < 27 else 2
                    while len(scored) > depth:
                        attend_pv(*scored.pop(0))
                if tail and not pend:
                    while scored:
                        attend_pv(*scored.pop(0))

            stt = {b: {"pend": list(range(NKB)), "qc": 0, "vc": 0}
                   for b in range(B)}

            def emit_v(ci, tail=False):
                t0, sz = CHUNKS[ci]
                b = t0 // L
                s = stt[b]
                proj_v(ci)
                s["vc"] = t0 - b * L + sz
                pop_ready(b, s["pend"], s["qc"], s["vc"], tail=tail)

            prev = []
            for ci in EMIT:
                t0, sz = CHUNKS[ci]
                b = t0 // L
                s = stt[b]
                if sz == 2 * G:
                    proj_qk2(ci, w_sb["wq8"], w_sb["wql"], bq_sb, qt_sb)
                    if len(prev) > 1:
                        emit_v(prev.pop(0))  # V lags two chunks
                    proj_qk2(ci, w_sb["wk8"], w_sb["wkl"], bk_sb, kt_sb)
                    s["qc"] = t0 - b * L + sz
                    pop_ready(b, s["pend"], s["qc"], s["vc"])
                else:
                    proj_qk(ci, 0, w_sb["wq8"], w_sb["wql"], bq_sb, qt_sb)
                    proj_qk(ci, 0, w_sb["wk8"], w_sb["wkl"], bk_sb, kt_sb)
                    s["qc"] = t0 - b * L + sz
                    pop_ready(b, s["pend"], s["qc"], s["vc"])
                    if len(prev) > 1:
                        emit_v(prev.pop(0))
                prev.append(ci)
            emit_v(prev.pop(0))
            emit_v(prev.pop(0), tail=True)
            for b in range(B):
                assert not stt[b]["pend"], (b, stt[b])
    nc.finalize()
    return nc


_NC = None


def _get_nc():
    global _NC
    if _NC is None:
        _NC = build_program()
    return _NC


def _masks():
    pk = np.arange(P)[:, None]
    f = np.arange(P)[None, :]
    mkd = (f >= pk).astype(np.float32)       # diag block: query >= key
    mkt = (f < pk).astype(np.float32)        # tail block: dist <= 255
    mkd2 = np.repeat(mkd[:, None, :], 2, axis=1).astype(ml_dtypes.bfloat16)
    mkt2 = np.repeat(mkt[:, None, :], 2, axis=1).astype(ml_dtypes.bfloat16)
    return np.ascontiguousarray(mkd2), np.ascontiguousarray(mkt2)


def _prepare_in_maps(inputs):
    hs = np.asarray(inputs["hidden_states"], np.float32)
    Wq = np.asarray(inputs["Wq"], np.float32)
    Wk = np.asarray(inputs["Wk"], np.float32)
    Wv = np.asarray(inputs["Wv"], np.float32)
    bq = np.asarray(inputs["bq"], np.float32)
    bk = np.asarray(inputs["bk"], np.float32)

    x_flat = hs.reshape(NT, D)
    # xt[p, k, t] = x_flat[t, k*128+p]
    xt = np.ascontiguousarray(
        x_flat.T.reshape(KSUB, P, NT).transpose(1, 0, 2))
    xh = xt.astype(ml_dtypes.float8_e4m3)
    xl = (xt - xh.astype(np.float32)).astype(ml_dtypes.float8_e4m3)
    chunks = {}
    for i, (t0, sz) in enumerate(CHUNKS):
        chunks[f"xh{i}"] = np.ascontiguousarray(xh[:, :, t0:t0 + sz])
        chunks[f"xl{i}"] = np.ascontiguousarray(xl[:, :, t0:t0 + sz])
    mkd, mkt = _masks()

    def wsplit(W, c):
        # [P, KSUB, 128]: w[p, k, m] = WS * W[k*128+p, c*128+m]
        ws = np.ascontiguousarray(
            (WS * W[:, c * P:(c + 1) * P]).reshape(KSUB, P, P)
            .transpose(1, 0, 2))
        w8 = ws.astype(ml_dtypes.float8_e4m3)
        wl = (ws - w8.astype(np.float32)).astype(ml_dtypes.float8_e4m3)
        return w8, wl

    in_maps = []
    for c in range(NCORES):
        wq8, wql = wsplit(Wq, c)
        wk8, wkl = wsplit(Wk, c)
        wvh, wvl = wsplit(Wv, c)
        bqc = np.ascontiguousarray(bq[c * P:(c + 1) * P].reshape(P, 1))
        bkc = np.ascontiguousarray(bk[c * P:(c + 1) * P].reshape(P, 1))
        qblob = np.concatenate(
            [w.reshape(P, KSUB * P).view(np.uint8)
             for w in (wq8, wql, wk8, wkl)]
            + [bqc.view(np.uint8), bkc.view(np.uint8)], axis=1)
        vblob = np.concatenate(
            [w.reshape(P, KSUB * P).view(np.uint8) for w in (wvh, wvl)]
            + [mkd.reshape(P, 2 * P).view(np.uint8),
               mkt.reshape(P, 2 * P).view(np.uint8)], axis=1)
        m = dict(chunks)
        m["qblob"] = np.ascontiguousarray(qblob)
        m["vblob"] = np.ascontiguousarray(vblob)
        in_maps.append(m)
    return in_maps


def run(inputs, trace=False, **kwargs):
    nc = _get_nc()
    in_maps = _prepare_in_maps(inputs)
    res = run_bass_kernel_spmd(nc, in_maps, core_ids=list(range(NCORES)),
                               trace=trace, **kwargs)
    bv = np.asarray(inputs["bv"], np.float32)
    outs = []
    for c in range(NCORES):
        o = np.asarray(res.results[c]["out"]).astype(np.float32)
        # [B, NSUP, P, 3, OC]; (s, slot) -> query block 3s+slot, row p
        o = o.reshape(B, NSUP, P, 3, OC).transpose(0, 1, 3, 2, 4)
        o = o.reshape(B, NSUP * 3 * P, OC)[:, :L]      # [B, L, OC]
        for h in range(2):
            c0 = h * (DH + 1)
            outs.append(o[:, :, c0:c0 + DH] / o[:, :, c0 + DH:c0 + DH + 1])
    full = np.concatenate(outs, axis=2)
    full = full + bv[None, None, :]
    return full.astype(np.float32), res


def kernel(**inputs):
    out, _ = run(inputs, trace=False)
    return out


# revision 64
# speedup vs baseline: 1.0161x; 1.0105x over previous
"""Local (sliding-window causal) attention kernel for Trainium2, 8 NeuronCores.

Reference computation (per batch b, head h):
  q = x @ Wq + bq ; k = x @ Wk + bk ; v = x @ Wv + bv   (16 heads of 64)
  S = q k^T / 8, masked to the causal band  i-255 <= j <= i
  out = softmax(S) @ v

Sharding: B=2, H=16 -> each of 8 cores owns a 128-wide column slice of the
QKV projections (2 heads) for both batches. Inputs are replicated; weights
column-sliced per core; no collectives.

Scheme (fp8 DoubleRow projections, bf16 attention):
  - x ships as an fp8 pair (xh = fp8(x^T), xl = fp8(x^T - xh)) in per-chunk
    tensors (contiguous rows -> 1 DMA descriptor per partition); weights as
    fp8 pairs of 64*W (64x scaling keeps W ~N(0,0.02) in e4m3 normal range).
    Projections accumulate correction terms in PSUM via DoubleRow
    (2 k-subtiles per pass):
       64*q = xh@wq8 + xh@wql [+ xl@wq8]     (same for k; v always 3 terms)
    then a tensor_scalar copy rescales by 1/64 (+bias) into bf16 SBUF.
  - Attention per (b, key-block kb of 128): S^T for both heads lands in one
    2-bank PSUM tile; one ACT exp (scale=1/8) -> P~^T bf16; the two
    triangular 0/1 masks multiply in (diag cols 0:128, tail cols 256:384;
    the middle 128 are always in-band) on DVE or Pool. PV matmuls
    accumulate [128q, 65] per (qb, h) into per-3-qb PSUM "super" tiles
    (col 64 = row sums via the ones-column of V'); a DVE copy stages
    [128, 3*130] bf16 to SBUF, shipped unnormalized; the host divides by
    the row sums and adds bv.
"""

import sys

import numpy as np

try:
    import concourse.bass as bass  # noqa: F401
except ImportError:
    sys.path.insert(0, "/opt/trn_rl_repo")

import concourse.bass as bass  # noqa: F401
import concourse.tile as tile
from concourse import bacc, mybir
from concourse.bass_utils import run_bass_kernel_spmd

import ml_dtypes

P = 128
B, L, D = 2, 2048, 1024
NT = B * L            # 4096 tokens
KSUB = D // P         # 8 contraction subtiles (4 DoubleRow pairs)
G = 256               # DoubleRow token group (rhs free = 2*G = 512)
NLB = NT // P         # 32 token blocks
NKB = L // P          # 16 key blocks per batch
QW = 384              # query window per key block
DH = 64               # head dim
OC = 2 * (DH + 1)     # output cols per token (2 heads x (o, rowsum))
NSUP = 6              # supers per batch (3 query blocks each)
NCORES = 8
WS = 64.0             # weight pre-scale for fp8
QK_TERMS = 2          # 3 = full correction, 2 = drop xl@w8 (faster, riskier)

# (start, size) of the x chunks; first two are small to cut startup latency
CHUNKS = [(0, 256), (256, 256), (512, 512), (1024, 512), (1536, 512),
          (2048, 512), (2560, 512), (3072, 512), (3584, 512)]

F32 = mybir.dt.float32
BF16 = mybir.dt.bfloat16
FP8 = mybir.dt.float8e4

DR = mybir.MatmulPerfMode.DoubleRow


def build_program():
    nc = bacc.Bacc("TRN2", target_bir_lowering=False, debug=False,
                   num_devices=NCORES)

    xh_ds, xl_ds = [], []
    for i, (t0, sz) in enumerate(CHUNKS):
        xh_ds.append(nc.dram_tensor(f"xh{i}", [P, KSUB, sz], FP8,
                                    kind="ExternalInput").ap())
        xl_ds.append(nc.dram_tensor(f"xl{i}", [P, KSUB, sz], FP8,
                                    kind="ExternalInput").ap())
    # constants ride in two DMAs: the QK blob (4 weight tensors + biases)
    # gates the first projection; the V blob (V weights + masks) only the
    # first attend.
    QBLOB = 4 * 1024 + 2 * 4
    VBLOB = 2 * 1024 + 2 * 512
    qblob_d = nc.dram_tensor("qblob", [P, QBLOB], mybir.dt.uint8,
                             kind="ExternalInput").ap()
    vblob_d = nc.dram_tensor("vblob", [P, VBLOB], mybir.dt.uint8,
                             kind="ExternalInput").ap()
    out_d = nc.dram_tensor("out", [B, NSUP, P, 3 * OC], BF16,
                           kind="ExternalOutput").ap()

    with tile.TileContext(nc) as tc:
        with (
            tc.tile_pool(name="const", bufs=1) as const,
            tc.tile_pool(name="qkv", bufs=1) as qkv,
            tc.tile_pool(name="xhp", bufs=4) as xhp,
            tc.tile_pool(name="xlp", bufs=4) as xlp,
            tc.tile_pool(name="ptp", bufs=12) as ptp,
            tc.tile_pool(name="ostp", bufs=3) as ostp,
            tc.tile_pool(name="pjps", bufs=2, space="PSUM") as pj_ps,
            tc.tile_pool(name="pvps", bufs=1, space="PSUM") as pv_ps,
            tc.tile_pool(name="stps", bufs=3, space="PSUM") as st_ps,
            tc.tile_pool(name="ops", bufs=2, space="PSUM") as o_ps,
        ):
            qblob = const.tile([P, QBLOB], mybir.dt.uint8, tag="qblob")
            vblob = const.tile([P, VBLOB], mybir.dt.uint8, tag="vblob")
            w_sb = {}
            for wi, wn in enumerate(("wq8", "wql", "wk8", "wkl")):
                w_sb[wn] = (qblob[:, wi * 1024:(wi + 1) * 1024]
                            .bitcast(FP8)
                            .rearrange("p (k m) -> p k m", k=KSUB))
            bq_sb = qblob[:, 4096:4100].bitcast(F32)
            bk_sb = qblob[:, 4100:4104].bitcast(F32)
            for wi, wn in enumerate(("wvh", "wvl")):
                w_sb[wn] = (vblob[:, wi * 1024:(wi + 1) * 1024]
                            .bitcast(FP8)
                            .rearrange("p (k m) -> p k m", k=KSUB))
            mkd_sb = (vblob[:, 2048:2560].bitcast(BF16)
                      .rearrange("p (h m) -> p h m", h=2))
            mkt_sb = (vblob[:, 2560:3072].bitcast(BF16)
                      .rearrange("p (h m) -> p h m", h=2))

            qt_sb = qkv.tile([P, NT], BF16, tag="qt")   # 2 heads' dh on parts
            kt_sb = qkv.tile([P, NT], BF16, tag="kt")
            v_sb = qkv.tile([P, 2, NLB, DH + 1], BF16, tag="v")
            nc.vector.memset(v_sb[:, :, :, DH:DH + 1], 1.0)

            EMIT = list(range(len(CHUNKS)))

            xhs, xls = {}, {}
            for j, i in enumerate(EMIT):
                sz = CHUNKS[i][1]
                xhs[i] = xhp.tile([P, KSUB, sz], FP8, tag=f"xh{j % 4}",
                                  name=f"xh{i}")
                xls[i] = xlp.tile([P, KSUB, sz], FP8, tag=f"xl{j % 4}",
                                  name=f"xl{i}")

            # xh leads xl by one chunk: Q/K only consume xh, and V (the only
            # xl consumer) is emitted one chunk behind.
            nc.sync.dma_start(qblob[:], qblob_d)
            nc.sync.dma_start(xhs[0][:], xh_ds[0])
            nc.sync.dma_start(xhs[1][:], xh_ds[1])
            nc.sync.dma_start(vblob[:], vblob_d)
            nc.sync.dma_start(xls[0][:], xl_ds[0])
            nc.sync.dma_start(xhs[2][:], xh_ds[2])
            for i in EMIT[3:]:
                nc.sync.dma_start(xhs[i][:], xh_ds[i])
                nc.sync.dma_start(xls[i - 2][:], xl_ds[i - 2])
            for i in EMIT[-2:]:
                nc.sync.dma_start(xls[i][:], xl_ds[i])

            # Two 256-col projection groups share each PSUM bank (the tile
            # tracker is region-level, and a start=True bank clear only
            # resets has_written -- finished data in the other half is
            # unaffected), giving 4 slots in 2 banks.
            pj_rot = {"tile": None, "half": 1}

            def pj_slot():
                if pj_rot["half"] == 1:
                    pj_rot["tile"] = pj_ps.tile([P, 2, G], F32, tag="pj",
                                                name="pj")
                    pj_rot["half"] = 0
                else:
                    pj_rot["half"] = 1
                return pj_rot["tile"][:, pj_rot["half"], :]

            def qk_mms(sl, ci, lg, w8, wl):
                t0, sz = CHUNKS[ci]
                g0 = lg * G
                terms = ((w8, xhs[ci]), (wl, xhs[ci]))
                if QK_TERMS == 3:
                    terms += ((w8, xls[ci]),)
                nmm = 4 * len(terms)
                n = 0
                for wt, xt in terms:
                    for kp in range(KSUB // 2):
                        nc.tensor.matmul(
                            sl, lhsT=wt[:, 2 * kp:2 * kp + 2, :],
                            rhs=xt[:, 2 * kp:2 * kp + 2, g0:g0 + G],
                            start=(n == 0), stop=(n == nmm - 1),
                            perf_mode=DR, skip_group_check=True)
                        n += 1

            def proj_qk2(ci, w8, wl, bias, dst):
                """Both 256-token groups of a 512 chunk; single copy."""
                t0, sz = CHUNKS[ci]
                for lg in range(2):
                    qk_mms(pj_slot(), ci, lg, w8, wl)
                full = pj_rot["tile"][:, :, :].rearrange("p a b -> p (a b)")
                nc.vector.tensor_scalar(
                    dst[:, t0:t0 + 2 * G], full,
                    1.0 / WS, bias[:, 0:1],
                    op0=mybir.AluOpType.mult, op1=mybir.AluOpType.add)

            def proj_qk(ci, lg, w8, wl, bias, dst, ceng="dve"):
                """One 256-token DoubleRow group for Q^T or K^T."""
                t0, sz = CHUNKS[ci]
                g0 = lg * G
                sl = pj_slot()
                terms = ((w8, xhs[ci]), (wl, xhs[ci]))
                if QK_TERMS == 3:
                    terms += ((w8, xls[ci]),)
                nmm = 4 * len(terms)
                n = 0
                for wt, xt in terms:
                    for kp in range(KSUB // 2):
                        nc.tensor.matmul(
                            sl, lhsT=wt[:, 2 * kp:2 * kp + 2, :],
                            rhs=xt[:, 2 * kp:2 * kp + 2, g0:g0 + G],
                            start=(n == 0), stop=(n == nmm - 1),
                            perf_mode=DR, skip_group_check=True)
                        n += 1
                if ceng == "act":
                    nc.scalar.activation(
                        dst[:, t0 + g0:t0 + g0 + G], sl,
                        mybir.ActivationFunctionType.Identity,
                        bias=bias[:, 0:1], scale=1.0 / WS)
                else:
                    nc.vector.tensor_scalar(
                        dst[:, t0 + g0:t0 + g0 + G], sl,
                        1.0 / WS, bias[:, 0:1],
                        op0=mybir.AluOpType.mult, op1=mybir.AluOpType.add)

            def proj_v(ci):
                """V for one chunk: one PSUM bank, 128-token lb blocks."""
                t0, sz = CHUNKS[ci]
                nlb = sz // P
                pv = pv_ps.tile([P, 4, P], F32, tag="pv", name="pv")
                terms = ((w_sb["wvh"], xhs[ci]), (w_sb["wvl"], xhs[ci]),
                         (w_sb["wvh"], xls[ci]))
                for lo in range(nlb):
                    n = 0
                    for wt, xt in terms:
                        for kp in range(KSUB // 2):
                            nc.tensor.matmul(
                                pv[:, lo, :],
                                lhsT=xt[:, 2 * kp:2 * kp + 2,
                                        lo * P:(lo + 1) * P],
                                rhs=wt[:, 2 * kp:2 * kp + 2, :],
                                start=(n == 0), stop=(n == 11),
                                perf_mode=DR, skip_group_check=True)
                            n += 1
                lb0 = t0 // P
                nc.vector.tensor_scalar_mul(
                    v_sb[:, :, lb0:lb0 + nlb, 0:DH],
                    pv[:, 0:nlb, :].rearrange("p l (h d) -> p h l d", h=2),
                    1.0 / WS)

            o_tiles = {}
            o_done = {}

            def flush_super(b, s):
                """Copy a finished 3-qb PSUM super tile to SBUF + DMA out."""
                nslots = min(3, NKB - 3 * s)
                ot = o_tiles.pop((b, s))
                st = ostp.tile([P, 3 * OC], BF16, tag="ost",
                               name=f"ost_{b}_{s}")
                w = nslots * OC
                src_ap = ot[:, 0:nslots, :].rearrange("p s c -> p (s c)")
                if b == 1 and s >= 4:
                    nc.scalar.copy(st[:, 0:w], src_ap)
                else:
                    nc.vector.tensor_copy(st[:, 0:w], src_ap)
                nc.sync.dma_start(out_d[b, s, :, 0:w], st[:, 0:w])

            def pv_block(b, kb, h, pt_ap):
                for qb in range(kb, min(kb + 3, NKB)):
                    s, slot = divmod(qb, 3)
                    qoff = (qb - kb) * P
                    first = (kb == max(qb - 2, 0))
                    if first and slot == 0 and h == 0:
                        o_tiles[(b, s)] = o_ps.tile(
                            [P, 3, OC], F32, tag="o", name=f"o_{b}_{s}")
                    ot = o_tiles[(b, s)]
                    nc.tensor.matmul(
                        ot[:, slot, h * (DH + 1):(h + 1) * (DH + 1)],
                        lhsT=pt_ap[:, qoff:qoff + P],
                        rhs=v_sb[:, h, b * NKB + kb, :],
                        start=(first and slot == 0 and h == 0),
                        stop=(qb == kb), skip_group_check=True)
                    if qb == kb and h == 1:
                        done = o_done.get((b, s), 0) + 1
                        o_done[(b, s)] = done
                        if done == min(3, NKB - 3 * s):
                            flush_super(b, s)

            def attend_scores(b, kb, mask_engine):
                """S^T + exp + masks for one key block; PV comes later."""
                t0 = b * L
                k0 = t0 + kb * P
                qw = min(QW, L - kb * P)
                eng = nc.gpsimd if mask_engine == "pool" else nc.vector
                pt = ptp.tile([P, 2, QW], BF16, tag="pt", name="pt")
                for h in range(2):
                    hs = h * DH
                    sth = st_ps.tile([P, 512], F32, tag="st", name=f"st{h}")
                    nc.tensor.matmul(sth[:, 0:qw],
                                     lhsT=kt_sb[hs:hs + DH, k0:k0 + P],
                                     rhs=qt_sb[hs:hs + DH, k0:k0 + qw],
                                     start=True, stop=True)
                    nc.scalar.activation(
                        pt[:, h, 0:qw], sth[:, 0:qw],
                        mybir.ActivationFunctionType.Exp, scale=0.125)
                eng.tensor_mul(pt[:, :, 0:P], pt[:, :, 0:P], mkd_sb[:])
                if qw == QW:
                    eng.tensor_mul(pt[:, :, 2 * P:3 * P],
                                   pt[:, :, 2 * P:3 * P], mkt_sb[:])
                return pt

            def attend_pv(b, kb, pt):
                for h in range(2):
                    pv_block(b, kb, h, pt[:, h, :])

            # Attend(b, kb) is ready once Q^T/K^T cover batch-local token
            # (kb+3)*128 and V covers key block kb.  Scores run one block
            # ahead of PV so the PE never waits on the exp+mask chain of
            # the block it just scored.
            att_i = [0]
            scored = []

            def pop_ready(b, pend, q_cover, v_cover, tail=False):
                while pend:
                    kb = pend[0]
                    if (min(kb + 3, NKB) * P > q_cover
                            or (kb + 1) * P > v_cover):
                        break
                    pend.pop(0)
                    i = att_i[0]
                    att_i[0] += 1
                    eng = "pool" if 6 <= i < 16 else "dve"
                    pt = attend_scores(b, kb, eng)
                    scored.append((b, kb, pt))
                    depth = 8 if att_i[0] < 27 else 2
                    while len(scored) > depth:
                        attend_pv(*scored.pop(0))
                if tail and not pend:
                    while scored:
                        attend_pv(*scored.pop(0))

            stt = {b: {"pend": list(range(NKB)), "qc": 0, "vc": 0}
                   for b in range(B)}

            def emit_v(ci, tail=False):
                t0, sz = CHUNKS[ci]
                b = t0 // L
                s = stt[b]
                proj_v(ci)
                s["vc"] = t0 - b * L + sz
                pop_ready(b, s["pend"], s["qc"], s["vc"], tail=tail)

            prev = []
            for ci in EMIT:
                t0, sz = CHUNKS[ci]
                b = t0 // L
                s = stt[b]
                if sz == 2 * G:
                    proj_qk2(ci, w_sb["wq8"], w_sb["wql"], bq_sb, qt_sb)
                    if len(prev) > 1:
                        emit_v(prev.pop(0))  # V lags two chunks
                    proj_qk2(ci, w_sb["wk8"], w_sb["wkl"], bk_sb, kt_sb)
                    s["qc"] = t0 - b * L + sz
                    pop_ready(b, s["pend"], s["qc"], s["vc"])
                else:
                    proj_qk(ci, 0, w_sb["wq8"], w_sb["wql"], bq_sb, qt_sb)
                    proj_qk(ci, 0, w_sb["wk8"], w_sb["wkl"], bk_sb, kt_sb)
                    s["qc"] = t0 - b * L + sz
                    pop_ready(b, s["pend"], s["qc"], s["vc"])
                    if len(prev) > 1:
                        emit_v(prev.pop(0))
                prev.append(ci)
            emit_v(prev.pop(0))
            emit_v(prev.pop(0), tail=True)
            for b in range(B):
                assert not stt[b]["pend"], (b, stt[b])
    nc.finalize()
    return nc


_NC = None


def _get_nc():
    global _NC
    if _NC is None:
        _NC = build_program()
    return _NC


def _masks():
    pk = np.arange(P)[:, None]
    f = np.arange(P)[None, :]
    mkd = (f >= pk).astype(np.float32)       # diag block: query >= key
    mkt = (f < pk).astype(np.float32)        # tail block: dist <= 255
    mkd2 = np.repeat(mkd[:, None, :], 2, axis=1).astype(ml_dtypes.bfloat16)
    mkt2 = np.repeat(mkt[:, None, :], 2, axis=1).astype(ml_dtypes.bfloat16)
    return np.ascontiguousarray(mkd2), np.ascontiguousarray(mkt2)


def _prepare_in_maps(inputs):
    hs = np.asarray(inputs["hidden_states"], np.float32)
    Wq = np.asarray(inputs["Wq"], np.float32)
    Wk = np.asarray(inputs["Wk"], np.float32)
    Wv = np.asarray(inputs["Wv"], np.float32)
    bq = np.asarray(inputs["bq"], np.float32)
    bk = np.asarray(inputs["bk"], np.float32)

    x_flat = hs.reshape(NT, D)
    # xt[p, k, t] = x_flat[t, k*128+p]
    xt = np.ascontiguousarray(
        x_flat.T.reshape(KSUB, P, NT).transpose(1, 0, 2))
    xh = xt.astype(ml_dtypes.float8_e4m3)
    xl = (xt - xh.astype(np.float32)).astype(ml_dtypes.float8_e4m3)
    chunks = {}
    for i, (t0, sz) in enumerate(CHUNKS):
        chunks[f"xh{i}"] = np.ascontiguousarray(xh[:, :, t0:t0 + sz])
        chunks[f"xl{i}"] = np.ascontiguousarray(xl[:, :, t0:t0 + sz])
    mkd, mkt = _masks()

    def wsplit(W, c):
        # [P, KSUB, 128]: w[p, k, m] = WS * W[k*128+p, c*128+m]
        ws = np.ascontiguousarray(
            (WS * W[:, c * P:(c + 1) * P]).reshape(KSUB, P, P)
            .transpose(1, 0, 2))
        w8 = ws.astype(ml_dtypes.float8_e4m3)
        wl = (ws - w8.astype(np.float32)).astype(ml_dtypes.float8_e4m3)
        return w8, wl

    in_maps = []
    for c in range(NCORES):
        wq8, wql = wsplit(Wq, c)
        wk8, wkl = wsplit(Wk, c)
        wvh, wvl = wsplit(Wv, c)
        bqc = np.ascontiguousarray(bq[c * P:(c + 1) * P].reshape(P, 1))
        bkc = np.ascontiguousarray(bk[c * P:(c + 1) * P].reshape(P, 1))
        qblob = np.concatenate(
            [w.reshape(P, KSUB * P).view(np.uint8)
             for w in (wq8, wql, wk8, wkl)]
            + [bqc.view(np.uint8), bkc.view(np.uint8)], axis=1)
        vblob = np.concatenate(
            [w.reshape(P, KSUB * P).view(np.uint8) for w in (wvh, wvl)]
            + [mkd.reshape(P, 2 * P).view(np.uint8),
               mkt.reshape(P, 2 * P).view(np.uint8)], axis=1)
        m = dict(chunks)
        m["qblob"] = np.ascontiguousarray(qblob)
        m["vblob"] = np.ascontiguousarray(vblob)
        in_maps.append(m)
    return in_maps


def run(inputs, trace=False, **kwargs):
    nc = _get_nc()
    in_maps = _prepare_in_maps(inputs)
    res = run_bass_kernel_spmd(nc, in_maps, core_ids=list(range(NCORES)),
                               trace=trace, **kwargs)
    bv = np.asarray(inputs["bv"], np.float32)
    outs = []
    for c in range(NCORES):
        o = np.asarray(res.results[c]["out"]).astype(np.float32)
        # [B, NSUP, P, 3, OC]; (s, slot) -> query block 3s+slot, row p
        o = o.reshape(B, NSUP, P, 3, OC).transpose(0, 1, 3, 2, 4)
        o = o.reshape(B, NSUP * 3 * P, OC)[:, :L]      # [B, L, OC]
        for h in range(2):
            c0 = h * (DH + 1)
            outs.append(o[:, :, c0:c0 + DH] / o[:, :, c0 + DH:c0 + DH + 1])
    full = np.concatenate(outs, axis=2)
    full = full + bv[None, None, :]
    return full.astype(np.float32), res


def kernel(**inputs):
    out, _ = run(inputs, trace=False)
    return out
